# revision 41
# baseline (speedup 1.0000x reference)
"""Trainium2 Bass kernel for nn_EstimatorQNN (10-qubit EstimatorQNN, B=2048).

Math: the RX(pi*x) encoding applied to |0...0> yields a product state
    psi_enc[b, k] = (-i)^popcount(k) * m[b, k]
with a REAL magnitude tensor  m[b, k] = prod_w (cos(pi x_bw/2) or sin(..)).
The weight-dependent circuit (Rot layers + CNOT rings) is a batch-independent
unitary U.  Folding the phases and the PauliZ sign into
    S = Re(V^H D V),  V = U diag(phase),  D = diag(+-1 by LSB)
(a real symmetric 1024x1024 matrix, precomputed on host from the 120 weight
scalars), the whole per-sample computation collapses to
    z_b = m_b^T S m_b.

Sharding: pure data parallel over 8 cores (256 samples each); S replicated.
S is symmetric, so only the 36 upper-triangle 128x128 blocks ship (bf16,
1.125MB, diagonal blocks pre-halved); the device computes
    y[:, block b] = sum_{a<=b} m[:, chunk a] @ S'_ab,   z = 2*sum_j m_j y_j.
Per core: m is built in bf16 from a host-shipped 256-column "prefix" over
wires 2..9 and a 4-column combo for wires 0,1, PE-transposed into k-major
chunks, accumulated on the tensor engine tracking the streamed S blocks,
and reduced with a DVE multiply + ACT accumulate (scale=2).
"""

import os

import ml_dtypes
import numpy as np

import concourse.bass as bass
import concourse.mybir as mybir
import concourse.tile as tile
from concourse.bass_utils import run_bass_kernel_spmd
from concourse.masks import make_identity

N_QUBITS = 10
N_LAYERS = 4
DIM = 2**N_QUBITS  # 1024
B = 2048
N_CORES = 8
B_CORE = B // N_CORES  # 256
F32 = mybir.dt.float32
F32R = mybir.dt.float32r
BF16 = mybir.dt.bfloat16

# ---------------------------------------------------------------------------
# Host-side precompute (weights -> S;  x -> prefix/combo)
# ---------------------------------------------------------------------------


def _apply_1q_np(state, gate, wire):
    b = state.shape[0]
    s = state.reshape(b, 2**wire, 2, -1)
    s = np.einsum("ij,bxjy->bxiy", gate, s)
    return s.reshape(b, DIM)


def _apply_cnot_np(state, ctrl, tgt):
    b = state.shape[0]
    s = state.reshape((b,) + (2,) * N_QUBITS)
    s0 = np.take(s, [0], axis=1 + ctrl)
    s1 = np.take(s, [1], axis=1 + ctrl)
    s1 = np.flip(s1, axis=1 + tgt)
    s = np.concatenate([s0, s1], axis=1 + ctrl)
    return s.reshape(b, DIM)


def _rot_matrix_np(phi, theta, omega):
    c = np.cos(theta / 2)
    s = np.sin(theta / 2)
    ep = np.exp(-0.5j * (phi + omega))
    em = np.exp(-0.5j * (phi - omega))
    return np.array(
        [[ep * c, -np.conj(em) * s], [em * s, np.conj(ep) * c]],
        dtype=np.complex128,
    )


def _build_S(weights):
    """S (1024,1024) float32 symmetric with z_b = m_b^T S m_b."""
    w = np.asarray(weights, dtype=np.float64)
    state = np.eye(DIM, dtype=np.complex128)  # rows become U|b>  => state = U^T
    for l in range(N_LAYERS):
        for q in range(N_QUBITS):
            g = _rot_matrix_np(w[l, q, 0], w[l, q, 1], w[l, q, 2])
            state = _apply_1q_np(state, g, q)
        r = (l % (N_QUBITS - 1)) + 1
        for q in range(N_QUBITS):
            state = _apply_cnot_np(state, q, (q + r) % N_QUBITS)

    k = np.arange(DIM)
    popcount = np.zeros(DIM, dtype=np.int64)
    for b in range(N_QUBITS):
        popcount += (k >> b) & 1
    phase = (-1j) ** (popcount % 4)
    sign = np.where(k % 2 == 0, 1.0, -1.0)

    V = (state * phase[:, None]).T  # V[k, j] = U[k, j] * phase[j]
    S = V.real.T @ (sign[:, None] * V.real) + V.imag.T @ (sign[:, None] * V.imag)
    return np.ascontiguousarray(S, dtype=np.float32)


def _build_enc(x):
    """enc (B, 260) f32: [prefix (256 cols, wires 2..9) | combo (4 cols, wires 0,1)].

    m[b, t*256 + r] = prefix[b, r] * combo[b, t];  r bit (9-w) is wire w.
    """
    ang = 0.5 * np.pi * np.asarray(x, dtype=np.float64)
    c, s = np.cos(ang), np.sin(ang)
    f = np.stack([c, s], axis=-1)  # (B, 10, 2)
    bsz = f.shape[0]
    prefix = np.ones((bsz, 1), dtype=np.float64)
    for w in range(2, N_QUBITS):
        prefix = np.einsum("br,bi->bri", prefix, f[:, w, :]).reshape(bsz, -1)
    combo = np.einsum("bi,bj->bij", f[:, 0, :], f[:, 1, :]).reshape(bsz, 4)
    return np.ascontiguousarray(
        np.concatenate([prefix, combo], axis=1), dtype=np.float32
    )


# ---------------------------------------------------------------------------
# Bass kernel (per core: 256 samples = 2 partition tiles of 128)
# ---------------------------------------------------------------------------

def _split_multi_waits(nc):
    """This image's walrus allows only ONE semaphore wait per engine
    instruction (setupSyncWait: "Too many sync wait commands").  Engine
    queues execute in order, so any extra waits can be peeled onto NoOp
    carrier instructions inserted immediately before the real one."""
    for bb in nc.main_func.blocks:
        insts = list(bb.instructions)
        out = []
        changed = False
        for ins in insts:
            si = ins.sync_info
            if si is not None and si.on_wait and len(si.on_wait) > 1:
                waits = list(si.on_wait)
                for w in waits[:-1]:
                    nop = mybir.InstNoOp(name=nc.get_next_instruction_name())
                    nop.engine = ins.engine
                    nop.sync_info = mybir.SyncInfo(on_wait=[w], on_update=[])
                    out.append(nop)
                ins.sync_info = mybir.SyncInfo(
                    on_wait=[waits[-1]], on_update=list(si.on_update or [])
                )
                changed = True
            out.append(ins)
        if changed:
            bb.instructions = out


class _Bass(bass.Bass):
    """This image's walrus rejects the gpsimd sem_clear ISA op (opcode 176,
    "ISA wrong length") that TileContext emits in its teardown.  The kernel
    executes once per NEFF load, so skipping the semaphore range-clear is
    safe; the DMA drain is kept."""

    def clear_and_free_semaphores(self, sems):
        if not sems:
            return
        sem_nums = [
            s.num if isinstance(s, bass.SemaphoreHandle) else s for s in sems
        ]
        for sem_range in bass.compact_to_ranges(sem_nums):
            assert self._state.free_isdisjoint(sem_range)
            self.gpsimd.dma_reset(sem_range)
        self._state.prepend_free_semaphores(sem_nums)
        for poison_set in self._tile_sem_poison_stack:
            poison_set.update(sem_nums)


N_TILES = 2  # sample tiles per core
N_CHUNKS = 8  # k chunks of 128
N_BANKS = 2  # psum banks of 512 columns


def _build_nc():
    nc = _Bass()
    enc_h = nc.dram_tensor("enc", [B_CORE, 260], F32, kind="ExternalInput")
    smat_h = nc.dram_tensor("stri", [128, 36 * 128], BF16, kind="ExternalInput")
    z_h = nc.dram_tensor("z", [B_CORE, 1], F32, kind="ExternalOutput")

    with tile.TileContext(nc) as tc, tc.tile_pool(name="main", bufs=1) as pool, \
            tc.tile_pool(name="tp_psum", bufs=4, space="PSUM") as tp_pool, \
            tc.tile_pool(name="acc_psum", bufs=1, space="PSUM") as acc_pool, \
            tc.tile_pool(name="scratch", bufs=2) as scratch_pool:
        # --- enc first on the sync ring: FIFO puts it ahead of the S bulk ---
        enc_t = []
        for t in range(N_TILES):
            e = pool.tile([128, 260], F32, tag=f"enc{t}", name=f"enc{t}")
            nc.sync.dma_start(out=e[:], in_=enc_h[t * 128 : (t + 1) * 128, :])
            enc_t.append(e)

        # --- S upper-triangle blocks (a<=b, diag pre-halved, idx=b(b+1)/2+a):
        # 1.125MB in three DMAs, in accumulation order. ---
        s_tri = pool.tile([128, 36 * 128], BF16, tag="stri", name="stri")
        for lo, hi in ((0, 10), (10, 21), (21, 36)):
            nc.sync.dma_start(
                out=s_tri[:, lo * 128 : hi * 128],
                in_=smat_h[:, lo * 128 : hi * 128],
            )

        # --- m build directly in bf16 (DVE); bf16 keeps rel err ~4e-3 ---
        mb_t = []
        for t in range(N_TILES):
            mb = pool.tile([128, DIM], BF16, tag=f"mb{t}", name=f"mb{t}")
            mb_t.append(mb)

        def emit_build(t):
            for q in range(4):
                nc.vector.tensor_scalar_mul(
                    mb_t[t][:, q * 256 : (q + 1) * 256],
                    enc_t[t][:, 0:256],
                    enc_t[t][:, 256 + q : 257 + q],
                )

        # identity in bf16 for the transposes
        ident_b = pool.tile([128, 128], BF16, tag="ident_b", name="ident_b")
        make_identity(nc, ident_b[:])

        mt = [
            pool.tile([128, B_CORE], BF16, tag=f"mt{c}", name=f"mt{c}")
            for c in range(N_CHUNKS)
        ]
        acc = [
            acc_pool.tile([128, DIM], F32, tag=f"acc{t}", name=f"acc{t}")
            for t in range(N_TILES)
        ]

        def emit_tp(t, c):
            tp = tp_pool.tile([128, 128], BF16, tag="tp", name="tp")
            nc.tensor.transpose(
                tp[:], mb_t[t][:, c * 128 : (c + 1) * 128], ident_b[:]
            )
            nc.vector.tensor_copy(mt[c][:, t * 128 : (t + 1) * 128], tp[:])

        def emit_mm(t, b):
            # y[:, block b] = sum_{a<=b} m[:, chunk a] @ S'_ab
            for a in range(b + 1):
                idx = b * (b + 1) // 2 + a
                nc.tensor.matmul(
                    acc[t][:, b * 128 : (b + 1) * 128],
                    lhsT=mt[a][:, t * 128 : (t + 1) * 128],
                    rhs=s_tri[:, idx * 128 : (idx + 1) * 128],
                    start=(a == 0),
                    stop=(a == b),
                )

        emit_build(0)
        for c in range(N_CHUNKS):
            emit_tp(0, c)
        emit_build(1)
        emit_mm(0, 0)
        for c in range(N_CHUNKS):
            emit_tp(1, c)
        for b in range(1, N_CHUNKS):
            emit_mm(0, b)
            emit_mm(1, b - 1)
        emit_mm(1, N_CHUNKS - 1)

        # --- z_b = sum_j acc[b, j] * m[b, j]: DVE mul, ACT accum-reduce ---
        zcomb = pool.tile([128, N_TILES], F32, tag="zcomb", name="zcomb")
        for t in range(N_TILES):
            sc = scratch_pool.tile([128, DIM], F32, tag="zprod", name="zprod")
            nc.vector.tensor_mul(sc[:], acc[t][:], mb_t[t][:])
            nc.scalar.activation(
                sc[:],
                sc[:],
                mybir.ActivationFunctionType.Copy,
                scale=2.0,
                accum_out=zcomb[:, t : t + 1],
            )
        z_view = z_h[:].rearrange("(t p) o -> p (t o)", t=N_TILES)
        nc.sync.dma_start(out=z_view, in_=zcomb[:])

    _split_multi_waits(nc)
    return nc


_NC_CACHE = None
LAST_RESULT = None


def kernel(inputs: np.ndarray, weights: np.ndarray) -> np.ndarray:
    global _NC_CACHE, LAST_RESULT
    x = np.asarray(inputs, dtype=np.float32)
    w = np.asarray(weights, dtype=np.float32)
    assert x.shape == (B, N_QUBITS)

    S = _build_S(w).astype(np.float64)
    for a in range(8):
        S[a * 128 : (a + 1) * 128, a * 128 : (a + 1) * 128] *= 0.5
    stri = np.concatenate(
        [
            S[a * 128 : (a + 1) * 128, b * 128 : (b + 1) * 128]
            for b in range(8)
            for a in range(b + 1)
        ],
        axis=1,
    ).astype(ml_dtypes.bfloat16)  # (128, 36*128), idx = b(b+1)/2 + a
    enc = _build_enc(x)

    if _NC_CACHE is None:
        _NC_CACHE = _build_nc()
    nc = _NC_CACHE

    in_maps = [
        {
            "enc": np.ascontiguousarray(enc[i * B_CORE : (i + 1) * B_CORE]),
            "stri": stri,
        }
        for i in range(N_CORES)
    ]
    res = run_bass_kernel_spmd(nc, in_maps, core_ids=list(range(N_CORES)))
    LAST_RESULT = res
    z = np.concatenate([r["z"] for r in res.results], axis=0)
    return z.astype(np.float32)


# revision 42
# speedup vs baseline: 1.0163x; 1.0163x over previous
"""Trainium2 Bass kernel for nn_EstimatorQNN (10-qubit EstimatorQNN, B=2048).

Math: the RX(pi*x) encoding applied to |0...0> yields a product state
    psi_enc[b, k] = (-i)^popcount(k) * m[b, k]
with a REAL magnitude tensor  m[b, k] = prod_w (cos(pi x_bw/2) or sin(..)).
The weight-dependent circuit (Rot layers + CNOT rings) is a batch-independent
unitary U.  Folding the phases and the PauliZ sign into
    S = Re(V^H D V),  V = U diag(phase),  D = diag(+-1 by LSB)
(a real symmetric 1024x1024 matrix, precomputed on host from the 120 weight
scalars), the whole per-sample computation collapses to
    z_b = m_b^T S m_b.

Sharding: pure data parallel over 8 cores (256 samples each); S replicated.
S is symmetric, so only the 36 upper-triangle 128x128 blocks ship (bf16,
1.125MB, diagonal blocks pre-halved); the device computes
    y[:, block b] = sum_{a<=b} m[:, chunk a] @ S'_ab,   z = 2*sum_j m_j y_j.
Per core: m is built in bf16 from a host-shipped 256-column "prefix" over
wires 2..9 and a 4-column combo for wires 0,1, PE-transposed into k-major
chunks, accumulated on the tensor engine tracking the streamed S blocks,
and reduced with a DVE multiply + ACT accumulate (scale=2).
"""

import os

import ml_dtypes
import numpy as np

import concourse.bass as bass
import concourse.mybir as mybir
import concourse.tile as tile
from concourse.bass_utils import run_bass_kernel_spmd
from concourse.masks import make_identity

N_QUBITS = 10
N_LAYERS = 4
DIM = 2**N_QUBITS  # 1024
B = 2048
N_CORES = 8
B_CORE = B // N_CORES  # 256
F32 = mybir.dt.float32
F32R = mybir.dt.float32r
BF16 = mybir.dt.bfloat16

# ---------------------------------------------------------------------------
# Host-side precompute (weights -> S;  x -> prefix/combo)
# ---------------------------------------------------------------------------


def _apply_1q_np(state, gate, wire):
    b = state.shape[0]
    s = state.reshape(b, 2**wire, 2, -1)
    s = np.einsum("ij,bxjy->bxiy", gate, s)
    return s.reshape(b, DIM)


def _apply_cnot_np(state, ctrl, tgt):
    b = state.shape[0]
    s = state.reshape((b,) + (2,) * N_QUBITS)
    s0 = np.take(s, [0], axis=1 + ctrl)
    s1 = np.take(s, [1], axis=1 + ctrl)
    s1 = np.flip(s1, axis=1 + tgt)
    s = np.concatenate([s0, s1], axis=1 + ctrl)
    return s.reshape(b, DIM)


def _rot_matrix_np(phi, theta, omega):
    c = np.cos(theta / 2)
    s = np.sin(theta / 2)
    ep = np.exp(-0.5j * (phi + omega))
    em = np.exp(-0.5j * (phi - omega))
    return np.array(
        [[ep * c, -np.conj(em) * s], [em * s, np.conj(ep) * c]],
        dtype=np.complex128,
    )


def _build_S(weights):
    """S (1024,1024) float32 symmetric with z_b = m_b^T S m_b."""
    w = np.asarray(weights, dtype=np.float64)
    state = np.eye(DIM, dtype=np.complex128)  # rows become U|b>  => state = U^T
    for l in range(N_LAYERS):
        for q in range(N_QUBITS):
            g = _rot_matrix_np(w[l, q, 0], w[l, q, 1], w[l, q, 2])
            state = _apply_1q_np(state, g, q)
        r = (l % (N_QUBITS - 1)) + 1
        for q in range(N_QUBITS):
            state = _apply_cnot_np(state, q, (q + r) % N_QUBITS)

    k = np.arange(DIM)
    popcount = np.zeros(DIM, dtype=np.int64)
    for b in range(N_QUBITS):
        popcount += (k >> b) & 1
    phase = (-1j) ** (popcount % 4)
    sign = np.where(k % 2 == 0, 1.0, -1.0)

    V = (state * phase[:, None]).T  # V[k, j] = U[k, j] * phase[j]
    S = V.real.T @ (sign[:, None] * V.real) + V.imag.T @ (sign[:, None] * V.imag)
    return np.ascontiguousarray(S, dtype=np.float32)


def _build_enc(x):
    """enc (B, 260) f32: [prefix (256 cols, wires 2..9) | combo (4 cols, wires 0,1)].

    m[b, t*256 + r] = prefix[b, r] * combo[b, t];  r bit (9-w) is wire w.
    """
    ang = 0.5 * np.pi * np.asarray(x, dtype=np.float64)
    c, s = np.cos(ang), np.sin(ang)
    f = np.stack([c, s], axis=-1)  # (B, 10, 2)
    bsz = f.shape[0]
    prefix = np.ones((bsz, 1), dtype=np.float64)
    for w in range(2, N_QUBITS):
        prefix = np.einsum("br,bi->bri", prefix, f[:, w, :]).reshape(bsz, -1)
    combo = np.einsum("bi,bj->bij", f[:, 0, :], f[:, 1, :]).reshape(bsz, 4)
    return np.ascontiguousarray(
        np.concatenate([prefix, combo], axis=1), dtype=np.float32
    )


# ---------------------------------------------------------------------------
# Bass kernel (per core: 256 samples = 2 partition tiles of 128)
# ---------------------------------------------------------------------------

def _split_multi_waits(nc):
    """This image's walrus allows only ONE semaphore wait per engine
    instruction (setupSyncWait: "Too many sync wait commands").  Engine
    queues execute in order, so any extra waits can be peeled onto NoOp
    carrier instructions inserted immediately before the real one."""
    for bb in nc.main_func.blocks:
        insts = list(bb.instructions)
        out = []
        changed = False
        for ins in insts:
            si = ins.sync_info
            if si is not None and si.on_wait and len(si.on_wait) > 1:
                waits = list(si.on_wait)
                for w in waits[:-1]:
                    nop = mybir.InstNoOp(name=nc.get_next_instruction_name())
                    nop.engine = ins.engine
                    nop.sync_info = mybir.SyncInfo(on_wait=[w], on_update=[])
                    out.append(nop)
                ins.sync_info = mybir.SyncInfo(
                    on_wait=[waits[-1]], on_update=list(si.on_update or [])
                )
                changed = True
            out.append(ins)
        if changed:
            bb.instructions = out


class _Bass(bass.Bass):
    """This image's walrus rejects the gpsimd sem_clear ISA op (opcode 176,
    "ISA wrong length") that TileContext emits in its teardown.  The kernel
    executes once per NEFF load, so skipping the semaphore range-clear is
    safe; the DMA drain is kept."""

    def clear_and_free_semaphores(self, sems):
        if not sems:
            return
        sem_nums = [
            s.num if isinstance(s, bass.SemaphoreHandle) else s for s in sems
        ]
        for sem_range in bass.compact_to_ranges(sem_nums):
            assert self._state.free_isdisjoint(sem_range)
            self.gpsimd.dma_reset(sem_range)
        self._state.prepend_free_semaphores(sem_nums)
        for poison_set in self._tile_sem_poison_stack:
            poison_set.update(sem_nums)


class _TileContext(tile.TileContext):
    """Trim the Tile epilogue: the stock teardown is drain -> barrier ->
    sem-clear -> barrier.  With the sem-clear gone (walrus can't encode it,
    see _Bass) the second barrier fences nothing, and the global-clock drain
    already observes every completion, so one barrier suffices."""

    def _drain_and_barrier(self, tick_clock, wait_clock):
        drain_inst = self.nc.sync.drain()
        wait_clock.add_sem_waits(
            drain_inst.ins, tile.ScopedClock({None: tick_clock.global_clock})
        )
        popped = self.nc._tile_sem_poison_stack.pop()
        assert popped is self._sem_poison
        self.nc.all_engine_barrier()


N_TILES = 2  # sample tiles per core
N_CHUNKS = 8  # k chunks of 128
N_BANKS = 2  # psum banks of 512 columns


def _build_nc():
    nc = _Bass()
    enc_h = nc.dram_tensor("enc", [B_CORE, 260], F32, kind="ExternalInput")
    smat_h = nc.dram_tensor("stri", [128, 36 * 128], BF16, kind="ExternalInput")
    z_h = nc.dram_tensor("z", [B_CORE, 1], F32, kind="ExternalOutput")

    with _TileContext(nc) as tc, tc.tile_pool(name="main", bufs=1) as pool, \
            tc.tile_pool(name="tp_psum", bufs=4, space="PSUM") as tp_pool, \
            tc.tile_pool(name="acc_psum", bufs=1, space="PSUM") as acc_pool, \
            tc.tile_pool(name="scratch", bufs=2) as scratch_pool:
        # --- enc first on the sync ring: FIFO puts it ahead of the S bulk ---
        enc_t = []
        for t in range(N_TILES):
            e = pool.tile([128, 260], F32, tag=f"enc{t}", name=f"enc{t}")
            nc.sync.dma_start(out=e[:], in_=enc_h[t * 128 : (t + 1) * 128, :])
            enc_t.append(e)

        # --- S upper-triangle blocks (a<=b, diag pre-halved, idx=b(b+1)/2+a):
        # 1.125MB in three DMAs, in accumulation order. ---
        s_tri = pool.tile([128, 36 * 128], BF16, tag="stri", name="stri")
        for lo, hi in ((0, 10), (10, 21), (21, 36)):
            nc.sync.dma_start(
                out=s_tri[:, lo * 128 : hi * 128],
                in_=smat_h[:, lo * 128 : hi * 128],
            )

        # --- m build directly in bf16 (DVE); bf16 keeps rel err ~4e-3 ---
        mb_t = []
        for t in range(N_TILES):
            mb = pool.tile([128, DIM], BF16, tag=f"mb{t}", name=f"mb{t}")
            mb_t.append(mb)

        def emit_build(t):
            for q in range(4):
                nc.vector.tensor_scalar_mul(
                    mb_t[t][:, q * 256 : (q + 1) * 256],
                    enc_t[t][:, 0:256],
                    enc_t[t][:, 256 + q : 257 + q],
                )

        # identity in bf16 for the transposes
        ident_b = pool.tile([128, 128], BF16, tag="ident_b", name="ident_b")
        make_identity(nc, ident_b[:])

        mt = [
            pool.tile([128, B_CORE], BF16, tag=f"mt{c}", name=f"mt{c}")
            for c in range(N_CHUNKS)
        ]
        acc = [
            acc_pool.tile([128, DIM], F32, tag=f"acc{t}", name=f"acc{t}")
            for t in range(N_TILES)
        ]

        def emit_tp(t, c):
            tp = tp_pool.tile([128, 128], BF16, tag="tp", name="tp")
            nc.tensor.transpose(
                tp[:], mb_t[t][:, c * 128 : (c + 1) * 128], ident_b[:]
            )
            nc.vector.tensor_copy(mt[c][:, t * 128 : (t + 1) * 128], tp[:])

        def emit_mm(t, b):
            # y[:, block b] = sum_{a<=b} m[:, chunk a] @ S'_ab
            for a in range(b + 1):
                idx = b * (b + 1) // 2 + a
                nc.tensor.matmul(
                    acc[t][:, b * 128 : (b + 1) * 128],
                    lhsT=mt[a][:, t * 128 : (t + 1) * 128],
                    rhs=s_tri[:, idx * 128 : (idx + 1) * 128],
                    start=(a == 0),
                    stop=(a == b),
                )

        emit_build(0)
        for c in range(N_CHUNKS):
            emit_tp(0, c)
        emit_build(1)
        emit_mm(0, 0)
        for c in range(N_CHUNKS):
            emit_tp(1, c)
        for b in range(1, N_CHUNKS):
            emit_mm(0, b)
            emit_mm(1, b - 1)
        emit_mm(1, N_CHUNKS - 1)

        # --- z_b = sum_j acc[b, j] * m[b, j]: DVE mul, ACT accum-reduce ---
        zcomb = pool.tile([128, N_TILES], F32, tag="zcomb", name="zcomb")
        for t in range(N_TILES):
            sc = scratch_pool.tile([128, DIM], F32, tag="zprod", name="zprod")
            nc.vector.tensor_mul(sc[:], acc[t][:], mb_t[t][:])
            nc.scalar.activation(
                sc[:],
                sc[:],
                mybir.ActivationFunctionType.Copy,
                scale=2.0,
                accum_out=zcomb[:, t : t + 1],
            )
        z_view = z_h[:].rearrange("(t p) o -> p (t o)", t=N_TILES)
        nc.sync.dma_start(out=z_view, in_=zcomb[:])

    _split_multi_waits(nc)
    return nc


_NC_CACHE = None
LAST_RESULT = None


def kernel(inputs: np.ndarray, weights: np.ndarray) -> np.ndarray:
    global _NC_CACHE, LAST_RESULT
    x = np.asarray(inputs, dtype=np.float32)
    w = np.asarray(weights, dtype=np.float32)
    assert x.shape == (B, N_QUBITS)

    S = _build_S(w).astype(np.float64)
    for a in range(8):
        S[a * 128 : (a + 1) * 128, a * 128 : (a + 1) * 128] *= 0.5
    stri = np.concatenate(
        [
            S[a * 128 : (a + 1) * 128, b * 128 : (b + 1) * 128]
            for b in range(8)
            for a in range(b + 1)
        ],
        axis=1,
    ).astype(ml_dtypes.bfloat16)  # (128, 36*128), idx = b(b+1)/2 + a
    enc = _build_enc(x)

    if _NC_CACHE is None:
        _NC_CACHE = _build_nc()
    nc = _NC_CACHE

    in_maps = [
        {
            "enc": np.ascontiguousarray(enc[i * B_CORE : (i + 1) * B_CORE]),
            "stri": stri,
        }
        for i in range(N_CORES)
    ]
    res = run_bass_kernel_spmd(nc, in_maps, core_ids=list(range(N_CORES)))
    LAST_RESULT = res
    z = np.concatenate([r["z"] for r in res.results], axis=0)
    return z.astype(np.float32)


# revision 44
# speedup vs baseline: 1.0300x; 1.0134x over previous
"""Trainium2 Bass kernel for nn_EstimatorQNN (10-qubit EstimatorQNN, B=2048).

Math: the RX(pi*x) encoding applied to |0...0> yields a product state
    psi_enc[b, k] = (-i)^popcount(k) * m[b, k]
with a REAL magnitude tensor  m[b, k] = prod_w (cos(pi x_bw/2) or sin(..)).
The weight-dependent circuit (Rot layers + CNOT rings) is a batch-independent
unitary U.  Folding the phases and the PauliZ sign into
    S = Re(V^H D V),  V = U diag(phase),  D = diag(+-1 by LSB)
(a real symmetric 1024x1024 matrix, precomputed on host from the 120 weight
scalars), the whole per-sample computation collapses to
    z_b = m_b^T S m_b.

Sharding: pure data parallel over 8 cores (256 samples each); S replicated.
S is symmetric, so only the 36 upper-triangle 128x128 blocks ship (bf16,
1.125MB, diagonal blocks pre-halved); the device computes
    y[:, block b] = sum_{a<=b} m[:, chunk a] @ S'_ab,   z = 2*sum_j m_j y_j.
Per core: m is built in bf16 from a host-shipped 256-column "prefix" over
wires 2..9 and a 4-column combo for wires 0,1, PE-transposed into k-major
chunks, accumulated on the tensor engine tracking the streamed S blocks,
and reduced with a DVE multiply + ACT accumulate (scale=2).
"""

import os

import ml_dtypes
import numpy as np

import concourse.bass as bass
import concourse.mybir as mybir
import concourse.tile as tile
from concourse.bass_utils import run_bass_kernel_spmd
from concourse.masks import make_identity

N_QUBITS = 10
N_LAYERS = 4
DIM = 2**N_QUBITS  # 1024
B = 2048
N_CORES = 8
B_CORE = B // N_CORES  # 256
F32 = mybir.dt.float32
F32R = mybir.dt.float32r
BF16 = mybir.dt.bfloat16

# ---------------------------------------------------------------------------
# Host-side precompute (weights -> S;  x -> prefix/combo)
# ---------------------------------------------------------------------------


def _apply_1q_np(state, gate, wire):
    b = state.shape[0]
    s = state.reshape(b, 2**wire, 2, -1)
    s = np.einsum("ij,bxjy->bxiy", gate, s)
    return s.reshape(b, DIM)


def _apply_cnot_np(state, ctrl, tgt):
    b = state.shape[0]
    s = state.reshape((b,) + (2,) * N_QUBITS)
    s0 = np.take(s, [0], axis=1 + ctrl)
    s1 = np.take(s, [1], axis=1 + ctrl)
    s1 = np.flip(s1, axis=1 + tgt)
    s = np.concatenate([s0, s1], axis=1 + ctrl)
    return s.reshape(b, DIM)


def _rot_matrix_np(phi, theta, omega):
    c = np.cos(theta / 2)
    s = np.sin(theta / 2)
    ep = np.exp(-0.5j * (phi + omega))
    em = np.exp(-0.5j * (phi - omega))
    return np.array(
        [[ep * c, -np.conj(em) * s], [em * s, np.conj(ep) * c]],
        dtype=np.complex128,
    )


def _build_S(weights):
    """S (1024,1024) float32 symmetric with z_b = m_b^T S m_b."""
    w = np.asarray(weights, dtype=np.float64)
    state = np.eye(DIM, dtype=np.complex128)  # rows become U|b>  => state = U^T
    for l in range(N_LAYERS):
        for q in range(N_QUBITS):
            g = _rot_matrix_np(w[l, q, 0], w[l, q, 1], w[l, q, 2])
            state = _apply_1q_np(state, g, q)
        r = (l % (N_QUBITS - 1)) + 1
        for q in range(N_QUBITS):
            state = _apply_cnot_np(state, q, (q + r) % N_QUBITS)

    k = np.arange(DIM)
    popcount = np.zeros(DIM, dtype=np.int64)
    for b in range(N_QUBITS):
        popcount += (k >> b) & 1
    phase = (-1j) ** (popcount % 4)
    sign = np.where(k % 2 == 0, 1.0, -1.0)

    V = (state * phase[:, None]).T  # V[k, j] = U[k, j] * phase[j]
    S = V.real.T @ (sign[:, None] * V.real) + V.imag.T @ (sign[:, None] * V.imag)
    return np.ascontiguousarray(S, dtype=np.float32)


def _build_enc(x):
    """enc (B, 260) f32: [prefix (256 cols, wires 2..9) | combo (4 cols, wires 0,1)].

    m[b, t*256 + r] = prefix[b, r] * combo[b, t];  r bit (9-w) is wire w.
    """
    ang = 0.5 * np.pi * np.asarray(x, dtype=np.float64)
    c, s = np.cos(ang), np.sin(ang)
    f = np.stack([c, s], axis=-1)  # (B, 10, 2)
    bsz = f.shape[0]
    prefix = np.ones((bsz, 1), dtype=np.float64)
    for w in range(2, N_QUBITS):
        prefix = np.einsum("br,bi->bri", prefix, f[:, w, :]).reshape(bsz, -1)
    combo = np.einsum("bi,bj->bij", f[:, 0, :], f[:, 1, :]).reshape(bsz, 4)
    return np.ascontiguousarray(
        np.concatenate([prefix, combo], axis=1), dtype=np.float32
    )


# ---------------------------------------------------------------------------
# Bass kernel (per core: 256 samples = 2 partition tiles of 128)
# ---------------------------------------------------------------------------

def _split_multi_waits(nc):
    """This image's walrus allows only ONE semaphore wait per engine
    instruction (setupSyncWait: "Too many sync wait commands").  Engine
    queues execute in order, so any extra waits can be peeled onto NoOp
    carrier instructions inserted immediately before the real one."""
    for bb in nc.main_func.blocks:
        insts = list(bb.instructions)
        out = []
        changed = False
        for ins in insts:
            si = ins.sync_info
            if si is not None and si.on_wait and len(si.on_wait) > 1:
                waits = list(si.on_wait)
                for w in waits[:-1]:
                    nop = mybir.InstNoOp(name=nc.get_next_instruction_name())
                    nop.engine = ins.engine
                    nop.sync_info = mybir.SyncInfo(on_wait=[w], on_update=[])
                    out.append(nop)
                ins.sync_info = mybir.SyncInfo(
                    on_wait=[waits[-1]], on_update=list(si.on_update or [])
                )
                changed = True
            out.append(ins)
        if changed:
            bb.instructions = out


class _Bass(bass.Bass):
    """This image's walrus rejects the gpsimd sem_clear ISA op (opcode 176,
    "ISA wrong length") that TileContext emits in its teardown.  The kernel
    executes once per NEFF load, so skipping the semaphore range-clear is
    safe; the DMA drain is kept."""

    def clear_and_free_semaphores(self, sems):
        if not sems:
            return
        sem_nums = [
            s.num if isinstance(s, bass.SemaphoreHandle) else s for s in sems
        ]
        for sem_range in bass.compact_to_ranges(sem_nums):
            assert self._state.free_isdisjoint(sem_range)
            self.gpsimd.dma_reset(sem_range)
        self._state.prepend_free_semaphores(sem_nums)
        for poison_set in self._tile_sem_poison_stack:
            poison_set.update(sem_nums)


class _TileContext(tile.TileContext):
    """Trim the Tile epilogue: the stock teardown is drain -> barrier ->
    sem-clear -> barrier.  With the sem-clear gone (walrus can't encode it,
    see _Bass) the second barrier fences nothing, and the global-clock drain
    already observes every completion, so one barrier suffices."""

    def _drain_and_barrier(self, tick_clock, wait_clock):
        drain_inst = self.nc.sync.drain()
        wait_clock.add_sem_waits(
            drain_inst.ins, tile.ScopedClock({None: tick_clock.global_clock})
        )
        popped = self.nc._tile_sem_poison_stack.pop()
        assert popped is self._sem_poison
        self.nc.all_engine_barrier()


N_TILES = 2  # sample tiles per core
N_CHUNKS = 8  # k chunks of 128
N_BANKS = 2  # psum banks of 512 columns


def _build_nc():
    nc = _Bass()
    enc_h = nc.dram_tensor("enc", [B_CORE, 264], BF16, kind="ExternalInput")
    smat_h = nc.dram_tensor("stri", [128, 36 * 128], BF16, kind="ExternalInput")
    z_h = nc.dram_tensor("z", [B_CORE, 1], F32, kind="ExternalOutput")

    with _TileContext(nc) as tc, tc.tile_pool(name="main", bufs=1) as pool, \
            tc.tile_pool(name="tp_psum", bufs=4, space="PSUM") as tp_pool, \
            tc.tile_pool(name="acc_psum", bufs=1, space="PSUM") as acc_pool, \
            tc.tile_pool(name="scratch", bufs=2) as scratch_pool:
        # --- enc first on the sync ring: FIFO puts it ahead of the S bulk ---
        enc_t = []
        for t in range(N_TILES):
            e = pool.tile([128, 264], BF16, tag=f"enc{t}", name=f"enc{t}")
            nc.sync.dma_start(out=e[:], in_=enc_h[t * 128 : (t + 1) * 128, :])
            enc_t.append(e)

        # --- S upper-triangle blocks (a<=b, diag pre-halved, idx=b(b+1)/2+a):
        # 1.125MB in three DMAs, in accumulation order. ---
        s_tri = pool.tile([128, 36 * 128], BF16, tag="stri", name="stri")
        for lo, hi in ((0, 10), (10, 21), (21, 36)):
            nc.sync.dma_start(
                out=s_tri[:, lo * 128 : hi * 128],
                in_=smat_h[:, lo * 128 : hi * 128],
            )

        # --- m build directly in bf16 (DVE); bf16 keeps rel err ~4e-3 ---
        mb_t = []
        for t in range(N_TILES):
            mb = pool.tile([128, DIM], BF16, tag=f"mb{t}", name=f"mb{t}")
            mb_t.append(mb)

        def emit_build(t):
            for q in range(4):
                nc.vector.tensor_scalar_mul(
                    mb_t[t][:, q * 256 : (q + 1) * 256],
                    enc_t[t][:, 0:256],
                    enc_t[t][:, 256 + 2 * q : 258 + 2 * q].bitcast(F32),
                )

        # identity in bf16 for the transposes
        ident_b = pool.tile([128, 128], BF16, tag="ident_b", name="ident_b")
        make_identity(nc, ident_b[:])

        mt = [
            pool.tile([128, B_CORE], BF16, tag=f"mt{c}", name=f"mt{c}")
            for c in range(N_CHUNKS)
        ]
        acc = [
            acc_pool.tile([128, DIM], F32, tag=f"acc{t}", name=f"acc{t}")
            for t in range(N_TILES)
        ]

        def emit_tp(t, c):
            tp = tp_pool.tile([128, 128], BF16, tag="tp", name="tp")
            nc.tensor.transpose(
                tp[:], mb_t[t][:, c * 128 : (c + 1) * 128], ident_b[:]
            )
            nc.vector.tensor_copy(mt[c][:, t * 128 : (t + 1) * 128], tp[:])

        def emit_mm(t, b):
            # y[:, block b] = sum_{a<=b} m[:, chunk a] @ S'_ab
            for a in range(b + 1):
                idx = b * (b + 1) // 2 + a
                nc.tensor.matmul(
                    acc[t][:, b * 128 : (b + 1) * 128],
                    lhsT=mt[a][:, t * 128 : (t + 1) * 128],
                    rhs=s_tri[:, idx * 128 : (idx + 1) * 128],
                    start=(a == 0),
                    stop=(a == b),
                )

        emit_build(0)
        for c in range(N_CHUNKS):
            emit_tp(0, c)
        emit_build(1)
        emit_mm(0, 0)
        for c in range(N_CHUNKS):
            emit_tp(1, c)
        for b in range(1, N_CHUNKS):
            emit_mm(0, b)
            emit_mm(1, b - 1)
        emit_mm(1, N_CHUNKS - 1)

        # --- z_b = sum_j acc[b, j] * m[b, j]: DVE mul, ACT accum-reduce ---
        zcomb = pool.tile([128, N_TILES], F32, tag="zcomb", name="zcomb")
        for t in range(N_TILES):
            sc = scratch_pool.tile([128, DIM], F32, tag="zprod", name="zprod")
            nc.vector.tensor_mul(sc[:], acc[t][:], mb_t[t][:])
            nc.scalar.activation(
                sc[:],
                sc[:],
                mybir.ActivationFunctionType.Copy,
                scale=2.0,
                accum_out=zcomb[:, t : t + 1],
            )
        z_view = z_h[:].rearrange("(t p) o -> p (t o)", t=N_TILES)
        nc.sync.dma_start(out=z_view, in_=zcomb[:])

    _split_multi_waits(nc)
    return nc


_NC_CACHE = None
LAST_RESULT = None


def kernel(inputs: np.ndarray, weights: np.ndarray) -> np.ndarray:
    global _NC_CACHE, LAST_RESULT
    x = np.asarray(inputs, dtype=np.float32)
    w = np.asarray(weights, dtype=np.float32)
    assert x.shape == (B, N_QUBITS)

    S = _build_S(w).astype(np.float64)
    for a in range(8):
        S[a * 128 : (a + 1) * 128, a * 128 : (a + 1) * 128] *= 0.5
    stri = np.concatenate(
        [
            S[a * 128 : (a + 1) * 128, b * 128 : (b + 1) * 128]
            for b in range(8)
            for a in range(b + 1)
        ],
        axis=1,
    ).astype(ml_dtypes.bfloat16)  # (128, 36*128), idx = b(b+1)/2 + a
    enc = _build_enc(x)
    encb = np.zeros((B, 264), dtype=ml_dtypes.bfloat16)
    encb[:, 0:256] = enc[:, 0:256].astype(ml_dtypes.bfloat16)
    encb.view(np.uint16)[:, 256:264] = (
        np.ascontiguousarray(enc[:, 256:260]).view(np.uint16)
    )

    if _NC_CACHE is None:
        _NC_CACHE = _build_nc()
    nc = _NC_CACHE

    in_maps = [
        {
            "enc": np.ascontiguousarray(encb[i * B_CORE : (i + 1) * B_CORE]),
            "stri": stri,
        }
        for i in range(N_CORES)
    ]
    res = run_bass_kernel_spmd(nc, in_maps, core_ids=list(range(N_CORES)))
    LAST_RESULT = res
    z = np.concatenate([r["z"] for r in res.results], axis=0)
    return z.astype(np.float32)
